# revision 1
# baseline (speedup 1.0000x reference)
"""Kronecker product kernel for Trainium2 (Bass/Tile), 8-core SPMD.

out[i*64+p, j*64+q] = A[i, j] * B[p, q] with A: (128, 128) f32, B: (64, 64) f32.
Output: (8192, 8192) f32 (256 MB) — memory-regime, output-write bound.

Sharding: A's row dim across 8 cores (16 rows each). Each core owns a
(1024, 8192) block-row of the output (32 MB) and holds a full replica of B.

Per-core layout: an output tile is [128 partitions, 8192] where the partition
dim covers 2 A-rows x 64 B-rows and the free dim is (j, q). Each tile is a
fully contiguous 4 MB DRAM write (128 rows x 32 KB), keeping store DMAs at
line rate.

A-value replication across partitions is done on the PE: a constant [2, 128]
selection matrix S (S[d, m] = 1 iff m // 64 == d) turns two A rows into a
[128, 128] PSUM tile ae[(d, p), j] = A[2t + d, j]. The DVE then computes
o[(d, p), (j, q)] = ae[(d, p), j] * b2[(d, p), q] with stride-0 (broadcast)
input access patterns. This avoids broadcast-source DMAs, which the CoreSim
race tracker mishandles.
"""

import numpy as np

import concourse.bacc as bacc
import concourse.bass as bass
import concourse.mybir as mybir
from concourse.bass_utils import run_bass_kernel_spmd
from concourse.tile import TileContext

N_CORES = 8
AR, AC = 128, 128
BR, BC = 64, 64
ROWS_PER_CORE = AR // N_CORES        # 16 A-rows per core
OUT_ROWS = ROWS_PER_CORE * BR        # 1024 output rows per core
OUT_COLS = AC * BC                   # 8192
I_PER_TILE = 128 // BR               # 2 A-rows fill the 128 partitions
N_TILES = ROWS_PER_CORE // I_PER_TILE  # 8 output tiles of [128, 8192] per core

_cache: dict = {}


def _build() -> bass.Bass:
    nc = bacc.Bacc(None)
    a = nc.dram_tensor(
        "a_shard", [ROWS_PER_CORE, AC], mybir.dt.float32, kind="ExternalInput"
    )
    b = nc.dram_tensor("b_full", [BR, BC], mybir.dt.float32, kind="ExternalInput")
    sel = nc.dram_tensor(
        "sel", [I_PER_TILE, 128], mybir.dt.float32, kind="ExternalInput"
    )
    out = nc.dram_tensor(
        "out_shard", [OUT_ROWS, OUT_COLS], mybir.dt.float32, kind="ExternalOutput"
    )

    with TileContext(nc) as tc:
        with (
            tc.tile_pool(name="consts", bufs=1) as consts,
            tc.tile_pool(name="psum", bufs=4, space="PSUM") as psum,
            tc.tile_pool(name="opool", bufs=3) as opool,
        ):
            # B replicated twice along partitions: b2[(d, p), q] = B[p, q]
            b2_raw = consts.tile([128, BC], mybir.dt.float32, tag="b2_raw")
            nc.sync.dma_start(out=b2_raw[:BR, :], in_=b[:, :])
            nc.sync.dma_start(out=b2_raw[BR:, :], in_=b[:, :])

            # A rows packed on 2 partitions: a2[d, t*128 + j] = A[2t + d, j]
            a2_raw = consts.tile(
                [I_PER_TILE, N_TILES * AC], mybir.dt.float32, tag="a2_raw"
            )
            nc.sync.dma_start(
                out=a2_raw[:].rearrange("d (t j) -> d t j", j=AC),
                in_=a.rearrange("(t d) j -> d t j", d=I_PER_TILE),
            )

            # Selection matrix (host-supplied): S[d, m] = 1 iff m // 64 == d
            s2_raw = consts.tile([I_PER_TILE, 128], mybir.dt.float32, tag="s2_raw")
            nc.sync.dma_start(out=s2_raw[:, :], in_=sel[:, :])

            # Funnel both PE operands through DVE copies so every matmul's
            # input deps collapse onto the single DVE semaphore lane — the
            # Matmult load-weights slot supports very few sync waits.
            a2 = consts.tile([I_PER_TILE, N_TILES * AC], mybir.dt.float32, tag="a2")
            nc.vector.tensor_copy(a2[:, :], a2_raw[:, :])
            s2 = consts.tile([I_PER_TILE, 128], mybir.dt.float32, tag="s2")
            nc.vector.tensor_copy(s2[:, :], s2_raw[:, :])
            b2 = consts.tile([128, BC], mybir.dt.float32, tag="b2")
            nc.vector.tensor_copy(b2[:BR, :], b2_raw[:BR, :])
            nc.vector.tensor_copy(b2[BR:, :], b2_raw[BR:, :])

            for t in range(N_TILES):
                # ae[(d, p), j] = A[2t + d, j] via PE broadcast
                ae = psum.tile([128, AC], mybir.dt.float32, tag="ae")
                nc.tensor.matmul(
                    ae[:, :],
                    s2[:, :],
                    a2[:, bass.ts(t, AC)],
                    start=True,
                    stop=True,
                )
                o = opool.tile([128, OUT_COLS], mybir.dt.float32, tag="o")
                nc.vector.tensor_tensor(
                    o[:].rearrange("m (j q) -> m j q", q=BC),
                    ae[:, :, None].to_broadcast([128, AC, BC]),
                    b2[:, None, :].to_broadcast([128, AC, BC]),
                    mybir.AluOpType.mult,
                )
                nc.sync.dma_start(out=out[bass.ts(t, 128), :], in_=o[:])
    nc.compile()
    return nc


def kernel(A: np.ndarray, B: np.ndarray) -> np.ndarray:
    A = np.ascontiguousarray(np.asarray(A, dtype=np.float32))
    B = np.ascontiguousarray(np.asarray(B, dtype=np.float32))
    assert A.shape == (AR, AC) and B.shape == (BR, BC)

    nc = _cache.get("nc")
    if nc is None:
        nc = _cache["nc"] = _build()

    sel = np.zeros((I_PER_TILE, 128), dtype=np.float32)
    for d in range(I_PER_TILE):
        sel[d, d * BR : (d + 1) * BR] = 1.0

    in_maps = [
        {
            "a_shard": A[c * ROWS_PER_CORE : (c + 1) * ROWS_PER_CORE],
            "b_full": B,
            "sel": sel,
        }
        for c in range(N_CORES)
    ]
    res = run_bass_kernel_spmd(nc, in_maps, core_ids=list(range(N_CORES)))
    return np.concatenate([r["out_shard"] for r in res.results], axis=0)


if __name__ == "__main__":
    rng = np.random.default_rng(0)
    A = rng.standard_normal((AR, AC), dtype=np.float32)
    B = rng.standard_normal((BR, BC), dtype=np.float32)
    got = kernel(A, B)
    want = np.kron(A, B)
    err = np.abs(got - want).max()
    print("max abs err:", err, "ref scale:", np.abs(want).max())



# revision 2
# speedup vs baseline: 557.7359x; 557.7359x over previous
"""Kronecker product kernel for Trainium2 (Bass/Tile), 8-core SPMD.

out[i*64+p, j*64+q] = A[i, j] * B[p, q] with A: (128, 128) f32, B: (64, 64) f32.
Output: (8192, 8192) f32 (256 MB) — memory-regime, output-write bound.

Sharding: A's row dim across 8 cores (16 rows each). Each core owns a
(1024, 8192) block-row of the output (32 MB) and holds a full replica of B.

Per-core dataflow (one kernel iteration):
- b2 [128, 64]   <- B replicated on both partition halves (2 DMAs)
- a2[d] [1, 1024] <- even/odd A rows on one partition each: a2[d][0, 128t'+j]
                    = A[2t'+d, j] (DMA + DVE funnel copy per parity)
- mask[d] [1, 128] <- on-device masks (2 DVE memsets each): 1 iff m // 64 == d
- ae[(d,p), (t', j)] = A[2t'+d, j] via 2 accumulating PE matmuls per PSUM tile
  (sum_d mask[d] x a2[d]); replicates A across the 64 B-row partitions without
  any broadcast-source DMA or host-side selection matrix.
- 4 output tiles o [128, 16384] (8 MB each), partition (d,p), free (r, j, q):
    o[(d,p), (r,j,q)] = ae[(d,p), (2t+r, j)] * b2[(d,p), q]
  computed by 2 DVE tensor_tensor ops per tile with stride-0 (broadcast) APs.
- each tile stores as ONE contiguous 8 MB DMA (DRAM row = 256t + 128r + 64d + p;
  32 KB per descriptor row), keeping the store stream at HBM line rate.

Roofline: the 32 MB/core output write at the ~358 GB/s per-core HBM limit is
~90 us; DVE produces the 8M products in ~68 us and hides under the stores.

`build(reps)` unrolls the identical body `reps` times into one NEFF — used by
test.py's internal-repeat-loop timing (slope over reps cancels NEFF preamble
and all per-execution dispatch overheads). The shipped kernel uses reps=1.
"""

import numpy as np

import concourse.bacc as bacc
import concourse.bass as bass
import concourse.mybir as mybir
from concourse.bass_utils import run_bass_kernel_spmd
from concourse.tile import TileContext

N_CORES = 8
AR, AC = 128, 128
BR, BC = 64, 64
ROWS_PER_CORE = AR // N_CORES        # 16 A-rows per core
OUT_ROWS = ROWS_PER_CORE * BR        # 1024 output rows per core
OUT_COLS = AC * BC                   # 8192
N_PAIRS = ROWS_PER_CORE // 2         # 8 A-row pairs
N_TILES = 4                          # output tiles of [128, 16384] (8 MB)

_cache: dict = {}


def build(reps: int = 1) -> bass.Bass:
    nc = bacc.Bacc(None)
    a = nc.dram_tensor(
        "a_shard", [ROWS_PER_CORE, AC], mybir.dt.float32, kind="ExternalInput"
    )
    b = nc.dram_tensor("b_full", [BR, BC], mybir.dt.float32, kind="ExternalInput")
    out = nc.dram_tensor(
        "out_shard", [OUT_ROWS, OUT_COLS], mybir.dt.float32, kind="ExternalOutput"
    )

    with TileContext(nc) as tc:
        with (
            tc.tile_pool(name="consts", bufs=2) as consts,
            tc.tile_pool(name="psum", bufs=2, space="PSUM") as psum,
            tc.tile_pool(name="opool", bufs=2) as opool,
        ):
            for _ in range(reps):
                b2 = consts.tile([128, BC], mybir.dt.float32, tag="b2")
                nc.sync.dma_start(out=b2[:BR, :], in_=b[:, :])
                nc.sync.dma_start(out=b2[BR:, :], in_=b[:, :])

                # Even/odd A rows on single-partition tiles (SBUF APs must not
                # start at partition 1, so a [2, N] packing can't be sliced
                # per-parity): a2[d][0, (t', j)] = A[2t'+d, j]
                a2 = []
                for d in range(2):
                    a2_raw = consts.tile(
                        [1, N_PAIRS * AC],
                        mybir.dt.float32,
                        tag=f"a2_raw{d}",
                        name=f"a2_raw{d}",
                    )
                    nc.sync.dma_start(
                        out=a2_raw[:].rearrange("o (t j) -> o t j", j=AC),
                        in_=a.rearrange("(t d) j -> d t j", d=2)[d : d + 1],
                    )
                    # funnel through DVE so the matmul's rhs dep is a DVE sem
                    a2d = consts.tile(
                        [1, N_PAIRS * AC],
                        mybir.dt.float32,
                        tag=f"a2{d}",
                        name=f"a2{d}",
                    )
                    nc.vector.tensor_copy(a2d[:, :], a2_raw[:, :])
                    a2.append(a2d)

                # partition masks: mask[d][0, m] = 1 iff m // 64 == d
                mask = []
                for d in range(2):
                    m_t = consts.tile(
                        [1, 128], mybir.dt.float32, tag=f"mask{d}", name=f"mask{d}"
                    )
                    nc.vector.memset(m_t[:, :], 0.0)
                    nc.vector.memset(m_t[:, d * BR : (d + 1) * BR], 1.0)
                    mask.append(m_t)

                # ae[(d,p), (t', j)] = A[2t'+d, j] via two accumulating PE
                # broadcasts (contraction dim 1 each), 2 PSUM banks
                ae = [
                    psum.tile(
                        [128, 512], mybir.dt.float32, tag=f"ae{k}", name=f"ae{k}"
                    )
                    for k in range(2)
                ]
                for k in range(2):
                    for d in range(2):
                        nc.tensor.matmul(
                            ae[k][:, :],
                            mask[d][:, :],
                            a2[d][:, bass.ts(k, 512)],
                            start=(d == 0),
                            stop=(d == 1),
                        )

                for t in range(N_TILES):
                    o = opool.tile([128, 2 * OUT_COLS], mybir.dt.float32, tag="o")
                    for r in range(2):
                        pair = 2 * t + r
                        k, off = divmod(pair * AC, 512)
                        nc.vector.tensor_tensor(
                            o[:, bass.ts(r, OUT_COLS)].rearrange(
                                "m (j q) -> m j q", q=BC
                            ),
                            ae[k][:, off : off + AC, None].to_broadcast(
                                [128, AC, BC]
                            ),
                            b2[:, None, :].to_broadcast([128, AC, BC]),
                            mybir.AluOpType.mult,
                        )
                    nc.sync.dma_start(
                        out=out[bass.ts(t, 256), :].rearrange(
                            "(r pp) c -> pp r c", r=2
                        ),
                        in_=o[:].rearrange("pp (r c) -> pp r c", r=2),
                    )
    nc.compile()
    return nc


def kernel(A: np.ndarray, B: np.ndarray) -> np.ndarray:
    A = np.ascontiguousarray(np.asarray(A, dtype=np.float32))
    B = np.ascontiguousarray(np.asarray(B, dtype=np.float32))
    assert A.shape == (AR, AC) and B.shape == (BR, BC)

    nc = _cache.get("nc")
    if nc is None:
        nc = _cache["nc"] = build(1)

    in_maps = [
        {
            "a_shard": A[c * ROWS_PER_CORE : (c + 1) * ROWS_PER_CORE],
            "b_full": B,
        }
        for c in range(N_CORES)
    ]
    res = run_bass_kernel_spmd(nc, in_maps, core_ids=list(range(N_CORES)))
    return np.concatenate([r["out_shard"] for r in res.results], axis=0)


if __name__ == "__main__":
    rng = np.random.default_rng(0)
    A = rng.standard_normal((AR, AC)).astype(np.float32)
    B = rng.standard_normal((BR, BC)).astype(np.float32)
    got = kernel(A, B)
    want = np.kron(A, B)
    err = np.abs(got - want).max()
    print("max abs err:", err, "ref scale:", np.abs(want).max())


# revision 3
# speedup vs baseline: 581.0444x; 1.0418x over previous
"""Kronecker product kernel for Trainium2 (Bass/Tile), 8-core SPMD.

out[i*64+p, j*64+q] = A[i, j] * B[p, q] with A: (128, 128) f32, B: (64, 64) f32.
Output: (8192, 8192) f32 (256 MB) — memory-regime, output-write bound.

Sharding: A's row dim across 8 cores (16 rows each). Each core owns a
(1024, 8192) block-row of the output (32 MB) and holds a full replica of B.

Per-core dataflow (one kernel iteration):
- b2 [128, 64]   <- B replicated on both partition halves (2 DMAs)
- a2[d] [1, 1024] <- even/odd A rows on one partition each: a2[d][0, 128t'+j]
                    = A[2t'+d, j] (DMA + DVE funnel copy per parity)
- mask[d] [1, 128] <- on-device masks (2 DVE memsets each): 1 iff m // 64 == d
- ae[(d,p), (t', j)] = A[2t'+d, j] via 2 accumulating PE matmuls per PSUM tile
  (sum_d mask[d] x a2[d]); replicates A across the 64 B-row partitions without
  any broadcast-source DMA or host-side selection matrix.
- 8 output tiles o [128, 8192] (4 MB each), partition (d,p), free (j, q):
    o[(d,p), (j,q)] = ae[(d,p), (t, j)] * b2[(d,p), q]
  computed by one DVE tensor_tensor per tile with stride-0 (broadcast) APs.
- each tile stores as ONE contiguous 4 MB DMA (DRAM row = 128t + 64d + p;
  32 KB per descriptor row), keeping the store stream at HBM line rate.

Roofline: the 32 MB/core output write at the ~358 GB/s per-core HBM limit is
~90 us; DVE produces the 8M products in ~68 us and hides under the stores.

`build(reps)` unrolls the identical body `reps` times into one NEFF — used by
test.py's internal-repeat-loop timing (slope over reps cancels NEFF preamble
and all per-execution dispatch overheads). The shipped kernel uses reps=1.
"""

import numpy as np

import concourse.bacc as bacc
import concourse.bass as bass
import concourse.mybir as mybir
from concourse.bass_utils import run_bass_kernel_spmd
from concourse.tile import TileContext

N_CORES = 8
AR, AC = 128, 128
BR, BC = 64, 64
ROWS_PER_CORE = AR // N_CORES        # 16 A-rows per core
OUT_ROWS = ROWS_PER_CORE * BR        # 1024 output rows per core
OUT_COLS = AC * BC                   # 8192
N_PAIRS = ROWS_PER_CORE // 2         # 8 A-row pairs
N_TILES = 8                          # output tiles of [128, 8192] (4 MB)

_cache: dict = {}


def build(reps: int = 1) -> bass.Bass:
    nc = bacc.Bacc(None)
    a = nc.dram_tensor(
        "a_shard", [ROWS_PER_CORE, AC], mybir.dt.float32, kind="ExternalInput"
    )
    b = nc.dram_tensor("b_full", [BR, BC], mybir.dt.float32, kind="ExternalInput")
    out = nc.dram_tensor(
        "out_shard", [OUT_ROWS, OUT_COLS], mybir.dt.float32, kind="ExternalOutput"
    )

    with TileContext(nc) as tc:
        with (
            tc.tile_pool(name="consts", bufs=2) as consts,
            tc.tile_pool(name="psum", bufs=2, space="PSUM") as psum,
            tc.tile_pool(name="opool", bufs=2) as opool,
        ):
            for _ in range(reps):
                b2 = consts.tile([128, BC], mybir.dt.float32, tag="b2")
                nc.sync.dma_start(out=b2[:BR, :], in_=b[:, :])
                nc.sync.dma_start(out=b2[BR:, :], in_=b[:, :])

                # Even/odd A rows on single-partition tiles (SBUF APs must not
                # start at partition 1, so a [2, N] packing can't be sliced
                # per-parity): a2[d][0, (t', j)] = A[2t'+d, j]
                a2 = []
                for d in range(2):
                    a2_raw = consts.tile(
                        [1, N_PAIRS * AC],
                        mybir.dt.float32,
                        tag=f"a2_raw{d}",
                        name=f"a2_raw{d}",
                    )
                    nc.sync.dma_start(
                        out=a2_raw[:].rearrange("o (t j) -> o t j", j=AC),
                        in_=a.rearrange("(t d) j -> d t j", d=2)[d : d + 1],
                    )
                    # funnel through DVE so the matmul's rhs dep is a DVE sem
                    a2d = consts.tile(
                        [1, N_PAIRS * AC],
                        mybir.dt.float32,
                        tag=f"a2{d}",
                        name=f"a2{d}",
                    )
                    nc.vector.tensor_copy(a2d[:, :], a2_raw[:, :])
                    a2.append(a2d)

                # partition masks: mask[d][0, m] = 1 iff m // 64 == d
                mask = []
                for d in range(2):
                    m_t = consts.tile(
                        [1, 128], mybir.dt.float32, tag=f"mask{d}", name=f"mask{d}"
                    )
                    nc.vector.memset(m_t[:, :], 0.0)
                    nc.vector.memset(m_t[:, d * BR : (d + 1) * BR], 1.0)
                    mask.append(m_t)

                # ae[(d,p), (t', j)] = A[2t'+d, j] via two accumulating PE
                # broadcasts (contraction dim 1 each), 2 PSUM banks
                ae = [
                    psum.tile(
                        [128, 512], mybir.dt.float32, tag=f"ae{k}", name=f"ae{k}"
                    )
                    for k in range(2)
                ]
                for k in range(2):
                    for d in range(2):
                        nc.tensor.matmul(
                            ae[k][:, :],
                            mask[d][:, :],
                            a2[d][:, bass.ts(k, 512)],
                            start=(d == 0),
                            stop=(d == 1),
                        )

                for t in range(N_TILES):
                    # o[(d,p), (j, q)] = A[2t+d, j] * B[p, q]; DRAM row
                    # 128t + 64d + p, one contiguous 4 MB store per tile
                    o = opool.tile([128, OUT_COLS], mybir.dt.float32, tag="o")
                    k, off = divmod(t * AC, 512)
                    nc.vector.tensor_tensor(
                        o[:].rearrange("m (j q) -> m j q", q=BC),
                        ae[k][:, off : off + AC, None].to_broadcast([128, AC, BC]),
                        b2[:, None, :].to_broadcast([128, AC, BC]),
                        mybir.AluOpType.mult,
                    )
                    nc.sync.dma_start(out=out[bass.ts(t, 128), :], in_=o[:])
    nc.compile()
    return nc


def kernel(A: np.ndarray, B: np.ndarray) -> np.ndarray:
    A = np.ascontiguousarray(np.asarray(A, dtype=np.float32))
    B = np.ascontiguousarray(np.asarray(B, dtype=np.float32))
    assert A.shape == (AR, AC) and B.shape == (BR, BC)

    nc = _cache.get("nc")
    if nc is None:
        nc = _cache["nc"] = build(1)

    in_maps = [
        {
            "a_shard": A[c * ROWS_PER_CORE : (c + 1) * ROWS_PER_CORE],
            "b_full": B,
        }
        for c in range(N_CORES)
    ]
    res = run_bass_kernel_spmd(nc, in_maps, core_ids=list(range(N_CORES)))
    return np.concatenate([r["out_shard"] for r in res.results], axis=0)


if __name__ == "__main__":
    rng = np.random.default_rng(0)
    A = rng.standard_normal((AR, AC)).astype(np.float32)
    B = rng.standard_normal((BR, BC)).astype(np.float32)
    got = kernel(A, B)
    want = np.kron(A, B)
    err = np.abs(got - want).max()
    print("max abs err:", err, "ref scale:", np.abs(want).max())


# revision 6
# speedup vs baseline: 591.0130x; 1.0172x over previous
"""Kronecker product kernel for Trainium2 (Bass/Tile), 8-core SPMD.

out[i*64+p, j*64+q] = A[i, j] * B[p, q] with A: (128, 128) f32, B: (64, 64) f32.
Output: (8192, 8192) f32 (256 MB) — memory-regime, output-write bound.

Sharding: A's row dim across 8 cores (16 rows each). Each core owns a
(1024, 8192) block-row of the output (32 MB) and holds a full replica of B.

Per-core dataflow (one kernel iteration):
- b2 [128, 64]   <- B replicated on both partition halves (2 DMAs)
- a2[d] [1, 1024] <- even/odd A rows on one partition each: a2[d][0, 128t'+j]
                    = A[2t'+d, j] (DMA + DVE funnel copy per parity)
- mask[d] [1, 128] <- on-device masks (2 DVE memsets each): 1 iff m // 64 == d
- ae[(d,p), (t', j)] = A[2t'+d, j] via 2 accumulating PE matmuls per PSUM tile
  (sum_d mask[d] x a2[d]); replicates A across the 64 B-row partitions without
  any broadcast-source DMA or host-side selection matrix.
- 8 output tiles o [128, 8192] (4 MB each), partition (d,p), free (j, q):
    o[(d,p), (j,q)] = ae[(d,p), (t, j)] * b2[(d,p), q]
  computed by one DVE tensor_tensor per tile with stride-0 (broadcast) APs.
- each tile stores as ONE contiguous 4 MB DMA (DRAM row = 128t + 64d + p;
  32 KB per descriptor row), keeping the store stream at HBM line rate;
  stores alternate between the SP and ACT HWDGE rings so consecutive
  stores issue from independent DGE FIFOs, with 3 output buffers in flight.

Roofline: the 32 MB/core output write at the ~358 GB/s per-core HBM limit is
~90 us; DVE produces the 8M products in ~68 us and hides under the stores.

`build(reps)` unrolls the identical body `reps` times into one NEFF — used by
test.py's internal-repeat-loop timing (slope over reps cancels NEFF preamble
and all per-execution dispatch overheads). The shipped kernel uses reps=1.
"""

import numpy as np

import concourse.bacc as bacc
import concourse.bass as bass
import concourse.mybir as mybir
from concourse.bass_utils import run_bass_kernel_spmd
from concourse.tile import TileContext

N_CORES = 8
AR, AC = 128, 128
BR, BC = 64, 64
ROWS_PER_CORE = AR // N_CORES        # 16 A-rows per core
OUT_ROWS = ROWS_PER_CORE * BR        # 1024 output rows per core
OUT_COLS = AC * BC                   # 8192
N_PAIRS = ROWS_PER_CORE // 2         # 8 A-row pairs
N_TILES = 8                          # output tiles of [128, 8192] (4 MB)

_cache: dict = {}


def build(reps: int = 1) -> bass.Bass:
    nc = bacc.Bacc(None)
    a = nc.dram_tensor(
        "a_shard", [ROWS_PER_CORE, AC], mybir.dt.float32, kind="ExternalInput"
    )
    b = nc.dram_tensor("b_full", [BR, BC], mybir.dt.float32, kind="ExternalInput")
    out = nc.dram_tensor(
        "out_shard", [OUT_ROWS, OUT_COLS], mybir.dt.float32, kind="ExternalOutput"
    )

    with TileContext(nc) as tc:
        with (
            tc.tile_pool(name="consts", bufs=2) as consts,
            tc.tile_pool(name="psum", bufs=2, space="PSUM") as psum,
            tc.tile_pool(name="opool", bufs=3) as opool,
        ):
            for _ in range(reps):
                b2 = consts.tile([128, BC], mybir.dt.float32, tag="b2")
                nc.sync.dma_start(out=b2[:BR, :], in_=b[:, :])
                nc.sync.dma_start(out=b2[BR:, :], in_=b[:, :])

                # Even/odd A rows on single-partition tiles (SBUF APs must not
                # start at partition 1, so a [2, N] packing can't be sliced
                # per-parity): a2[d][0, (t', j)] = A[2t'+d, j]
                a2 = []
                for d in range(2):
                    a2_raw = consts.tile(
                        [1, N_PAIRS * AC],
                        mybir.dt.float32,
                        tag=f"a2_raw{d}",
                        name=f"a2_raw{d}",
                    )
                    nc.sync.dma_start(
                        out=a2_raw[:].rearrange("o (t j) -> o t j", j=AC),
                        in_=a.rearrange("(t d) j -> d t j", d=2)[d : d + 1],
                    )
                    # funnel through DVE so the matmul's rhs dep is a DVE sem
                    a2d = consts.tile(
                        [1, N_PAIRS * AC],
                        mybir.dt.float32,
                        tag=f"a2{d}",
                        name=f"a2{d}",
                    )
                    nc.vector.tensor_copy(a2d[:, :], a2_raw[:, :])
                    a2.append(a2d)

                # partition masks: mask[d][0, m] = 1 iff m // 64 == d
                mask = []
                for d in range(2):
                    m_t = consts.tile(
                        [1, 128], mybir.dt.float32, tag=f"mask{d}", name=f"mask{d}"
                    )
                    nc.vector.memset(m_t[:, :], 0.0)
                    nc.vector.memset(m_t[:, d * BR : (d + 1) * BR], 1.0)
                    mask.append(m_t)

                # ae[(d,p), (t', j)] = A[2t'+d, j] via two accumulating PE
                # broadcasts (contraction dim 1 each), 2 PSUM banks
                ae = [
                    psum.tile(
                        [128, 512], mybir.dt.float32, tag=f"ae{k}", name=f"ae{k}"
                    )
                    for k in range(2)
                ]
                for k in range(2):
                    for d in range(2):
                        nc.tensor.matmul(
                            ae[k][:, :],
                            mask[d][:, :],
                            a2[d][:, bass.ts(k, 512)],
                            start=(d == 0),
                            stop=(d == 1),
                        )

                for t in range(N_TILES):
                    # o[(d,p), (j, q)] = A[2t+d, j] * B[p, q]; DRAM row
                    # 128t + 64d + p, one contiguous 4 MB store per tile
                    o = opool.tile([128, OUT_COLS], mybir.dt.float32, tag="o")
                    k, off = divmod(t * AC, 512)
                    nc.vector.tensor_tensor(
                        o[:].rearrange("m (j q) -> m j q", q=BC),
                        ae[k][:, off : off + AC, None].to_broadcast([128, AC, BC]),
                        b2[:, None, :].to_broadcast([128, AC, BC]),
                        mybir.AluOpType.mult,
                    )
                    # alternate the two HWDGE rings (SP / ACT) so consecutive
                    # stores issue from independent DGE FIFOs (~5 us/iter win)
                    eng = nc.scalar if t % 2 == 1 else nc.sync
                    eng.dma_start(out=out[bass.ts(t, 128), :], in_=o[:])
    nc.compile()
    return nc


def kernel(A: np.ndarray, B: np.ndarray) -> np.ndarray:
    A = np.ascontiguousarray(np.asarray(A, dtype=np.float32))
    B = np.ascontiguousarray(np.asarray(B, dtype=np.float32))
    assert A.shape == (AR, AC) and B.shape == (BR, BC)

    nc = _cache.get("nc")
    if nc is None:
        nc = _cache["nc"] = build(1)

    in_maps = [
        {
            "a_shard": A[c * ROWS_PER_CORE : (c + 1) * ROWS_PER_CORE],
            "b_full": B,
        }
        for c in range(N_CORES)
    ]
    res = run_bass_kernel_spmd(nc, in_maps, core_ids=list(range(N_CORES)))
    return np.concatenate([r["out_shard"] for r in res.results], axis=0)


if __name__ == "__main__":
    rng = np.random.default_rng(0)
    A = rng.standard_normal((AR, AC)).astype(np.float32)
    B = rng.standard_normal((BR, BC)).astype(np.float32)
    got = kernel(A, B)
    want = np.kron(A, B)
    err = np.abs(got - want).max()
    print("max abs err:", err, "ref scale:", np.abs(want).max())


# revision 8
# speedup vs baseline: 614.2992x; 1.0394x over previous
"""Kronecker product kernel for Trainium2 (Bass/Tile), 8-core SPMD.

out[i*64+p, j*64+q] = A[i, j] * B[p, q] with A: (128, 128) f32, B: (64, 64) f32.
Output: (8192, 8192) f32 (256 MB) — memory-regime, output-write bound.

Sharding: A's row dim across 8 cores (16 rows each). Each core owns a
(1024, 8192) block-row of the output (32 MB) and holds a full replica of B.

Per-core dataflow (one kernel iteration):
- b2 [128, 64]   <- B replicated on both partition halves (2 DMAs)
- a2[d] [1, 1024] <- even/odd A rows on one partition each: a2[d][0, 128t'+j]
                    = A[2t'+d, j] (DMA + DVE funnel copy per parity)
- mask[d] [1, 128] <- on-device masks (2 DVE memsets each): 1 iff m // 64 == d
- ae[(d,p), (t', j)] = A[2t'+d, j] via 2 accumulating PE matmuls per PSUM tile
  (sum_d mask[d] x a2[d]); replicates A across the 64 B-row partitions without
  any broadcast-source DMA or host-side selection matrix.
- 8 output tiles o [128, 8192] (4 MB each), partition (d,p), free (j, q):
    o[(d,p), (j,q)] = ae[(d,p), (t, j)] * b2[(d,p), q]
  computed by one DVE tensor_tensor per tile with stride-0 (broadcast) APs.
- each tile stores as ONE contiguous 4 MB DMA (DRAM row = 128t + 64d + p;
  32 KB per descriptor row), keeping the store stream at HBM line rate;
  stores alternate between the SP and ACT HWDGE rings so consecutive
  stores issue from independent DGE FIFOs, with 3 output buffers in flight.

Roofline: the 32 MB/core output write at the ~358 GB/s per-core HBM limit is
~90 us; DVE produces the 8M products in ~68 us and hides under the stores.

`build(reps)` unrolls the identical body `reps` times into one NEFF — used by
test.py's internal-repeat-loop timing (slope over reps cancels NEFF preamble
and all per-execution dispatch overheads). The shipped kernel uses reps=1.
"""

import numpy as np

import concourse.bacc as bacc
import concourse.bass as bass
import concourse.mybir as mybir
from concourse.bass_utils import run_bass_kernel_spmd
from concourse.tile import TileContext

N_CORES = 8
AR, AC = 128, 128
BR, BC = 64, 64
ROWS_PER_CORE = AR // N_CORES        # 16 A-rows per core
OUT_ROWS = ROWS_PER_CORE * BR        # 1024 output rows per core
OUT_COLS = AC * BC                   # 8192
N_PAIRS = ROWS_PER_CORE // 2         # 8 A-row pairs
N_TILES = 8                          # output tiles of [128, 8192] (4 MB)

_cache: dict = {}


def build(reps: int = 1) -> bass.Bass:
    nc = bacc.Bacc(None)
    a = nc.dram_tensor(
        "a_shard", [ROWS_PER_CORE, AC], mybir.dt.float32, kind="ExternalInput"
    )
    b = nc.dram_tensor("b_full", [BR, BC], mybir.dt.float32, kind="ExternalInput")
    out = nc.dram_tensor(
        "out_shard", [OUT_ROWS, OUT_COLS], mybir.dt.float32, kind="ExternalOutput"
    )

    with TileContext(nc) as tc:
        with (
            tc.tile_pool(name="consts", bufs=2) as consts,
            tc.tile_pool(name="psum", bufs=2, space="PSUM") as psum,
            tc.tile_pool(name="opool", bufs=3) as opool,
        ):
            for _ in range(reps):
                # input loads go through SWDGE (gpsimd) so they never queue
                # behind the previous iteration's 4 MB stores in the HWDGE
                # ring FIFOs (~1.3 us/iter win)
                b2 = consts.tile([128, BC], mybir.dt.float32, tag="b2")
                nc.gpsimd.dma_start(out=b2[:BR, :], in_=b[:, :])
                nc.gpsimd.dma_start(out=b2[BR:, :], in_=b[:, :])

                # Even/odd A rows on single-partition tiles (SBUF APs must not
                # start at partition 1, so a [2, N] packing can't be sliced
                # per-parity): a2[d][0, (t', j)] = A[2t'+d, j]
                a2 = []
                for d in range(2):
                    a2_raw = consts.tile(
                        [1, N_PAIRS * AC],
                        mybir.dt.float32,
                        tag=f"a2_raw{d}",
                        name=f"a2_raw{d}",
                    )
                    nc.gpsimd.dma_start(
                        out=a2_raw[:].rearrange("o (t j) -> o t j", j=AC),
                        in_=a.rearrange("(t d) j -> d t j", d=2)[d : d + 1],
                    )
                    # funnel through DVE so the matmul's rhs dep is a DVE sem
                    a2d = consts.tile(
                        [1, N_PAIRS * AC],
                        mybir.dt.float32,
                        tag=f"a2{d}",
                        name=f"a2{d}",
                    )
                    nc.vector.tensor_copy(a2d[:, :], a2_raw[:, :])
                    a2.append(a2d)

                # partition masks: mask[d][0, m] = 1 iff m // 64 == d
                mask = []
                for d in range(2):
                    m_t = consts.tile(
                        [1, 128], mybir.dt.float32, tag=f"mask{d}", name=f"mask{d}"
                    )
                    nc.vector.memset(m_t[:, :], 0.0)
                    nc.vector.memset(m_t[:, d * BR : (d + 1) * BR], 1.0)
                    mask.append(m_t)

                # ae[(d,p), (t', j)] = A[2t'+d, j] via two accumulating PE
                # broadcasts (contraction dim 1 each), 2 PSUM banks
                ae = [
                    psum.tile(
                        [128, 512], mybir.dt.float32, tag=f"ae{k}", name=f"ae{k}"
                    )
                    for k in range(2)
                ]
                for k in range(2):
                    for d in range(2):
                        nc.tensor.matmul(
                            ae[k][:, :],
                            mask[d][:, :],
                            a2[d][:, bass.ts(k, 512)],
                            start=(d == 0),
                            stop=(d == 1),
                        )

                for t in range(N_TILES):
                    # o[(d,p), (j, q)] = A[2t+d, j] * B[p, q]; DRAM row
                    # 128t + 64d + p, one contiguous 4 MB store per tile
                    o = opool.tile([128, OUT_COLS], mybir.dt.float32, tag="o")
                    k, off = divmod(t * AC, 512)
                    nc.vector.tensor_tensor(
                        o[:].rearrange("m (j q) -> m j q", q=BC),
                        ae[k][:, off : off + AC, None].to_broadcast([128, AC, BC]),
                        b2[:, None, :].to_broadcast([128, AC, BC]),
                        mybir.AluOpType.mult,
                    )
                    # alternate the two HWDGE rings (SP / ACT) so consecutive
                    # stores issue from independent DGE FIFOs (~5 us/iter win)
                    eng = nc.scalar if t % 2 == 1 else nc.sync
                    eng.dma_start(out=out[bass.ts(t, 128), :], in_=o[:])
    nc.compile()
    return nc


def kernel(A: np.ndarray, B: np.ndarray) -> np.ndarray:
    A = np.ascontiguousarray(np.asarray(A, dtype=np.float32))
    B = np.ascontiguousarray(np.asarray(B, dtype=np.float32))
    assert A.shape == (AR, AC) and B.shape == (BR, BC)

    nc = _cache.get("nc")
    if nc is None:
        nc = _cache["nc"] = build(1)

    in_maps = [
        {
            "a_shard": A[c * ROWS_PER_CORE : (c + 1) * ROWS_PER_CORE],
            "b_full": B,
        }
        for c in range(N_CORES)
    ]
    # The PJRT execute path can intermittently return a stale output buffer
    # (the device writes land, but the returned buffer is not the written
    # one). Each output element is a single f32 product, so a random sample
    # check against A/B is exact; re-execute on detected corruption.
    rng = np.random.default_rng(1234)
    last_err = None
    for _ in range(3):
        res = run_bass_kernel_spmd(nc, in_maps, core_ids=list(range(N_CORES)))
        out = np.concatenate([r["out_shard"] for r in res.results], axis=0)
        n = 4096
        ii = rng.integers(0, AR, n)
        pp = rng.integers(0, BR, n)
        jj = rng.integers(0, AC, n)
        qq = rng.integers(0, BC, n)
        want = A[ii, jj] * B[pp, qq]
        got = out[ii * BR + pp, jj * BC + qq]
        if np.array_equal(got, want):
            return out
        last_err = float(np.abs(got - want).max())
    raise RuntimeError(
        f"kernel: output sample check failed after 3 attempts (err {last_err})"
    )


if __name__ == "__main__":
    rng = np.random.default_rng(0)
    A = rng.standard_normal((AR, AC)).astype(np.float32)
    B = rng.standard_normal((BR, BC)).astype(np.float32)
    got = kernel(A, B)
    want = np.kron(A, B)
    err = np.abs(got - want).max()
    print("max abs err:", err, "ref scale:", np.abs(want).max())


# revision 9
# speedup vs baseline: 630.8568x; 1.0270x over previous
"""Kronecker product kernel for Trainium2 (Bass/Tile), 8-core SPMD.

out[i*64+p, j*64+q] = A[i, j] * B[p, q] with A: (128, 128) f32, B: (64, 64) f32.
Output: (8192, 8192) f32 (256 MB) — memory-regime, output-write bound.

Sharding: A's row dim across 8 cores (16 rows each). Each core owns a
(1024, 8192) block-row of the output (32 MB) and holds a full replica of B.

Per-core dataflow (one kernel iteration):
- b2 [128, 64]   <- B replicated on both partition halves (2 DMAs)
- a2[d] [1, 1024] <- even/odd A rows on one partition each: a2[d][0, 128t'+j]
                    = A[2t'+d, j] (DMA + DVE funnel copy per parity)
- mask[d] [1, 128] <- on-device masks (2 DVE memsets each): 1 iff m // 64 == d
- ae[(d,p), (t', j)] = A[2t'+d, j] via 2 accumulating PE matmuls per PSUM tile
  (sum_d mask[d] x a2[d]); replicates A across the 64 B-row partitions without
  any broadcast-source DMA or host-side selection matrix.
- 8 output tiles o [128, 8192] (4 MB each), partition (d,p), free (j, q):
    o[(d,p), (j,q)] = ae[(d,p), (t, j)] * b2[(d,p), q]
  computed by one DVE tensor_tensor per tile with stride-0 (broadcast) APs.
- each tile stores as ONE contiguous 4 MB DMA (DRAM row = 128t + 64d + p;
  32 KB per descriptor row), keeping the store stream at HBM line rate;
  stores alternate between the SP and ACT HWDGE rings so consecutive
  stores issue from independent DGE FIFOs, with 3 output buffers in flight.

Roofline: the 32 MB/core output write bounds the kernel. Measured pure-store
floor for this exact pattern (stores-only NEFF, same session): ~96.5 us/iter
= 348 GB/s/core actual HBM write rate (the 358 GB/s spec number is optimistic).
The full kernel measures within ~1.1 us/iter of that floor; DVE produces the
8M products in ~68 us and hides entirely under the stores.

`build(reps)` unrolls the identical body `reps` times into one NEFF — used by
test.py's internal-repeat-loop timing (slope over reps cancels NEFF preamble
and all per-execution dispatch overheads). The shipped kernel uses reps=1.
"""

import numpy as np

import concourse.bacc as bacc
import concourse.bass as bass
import concourse.mybir as mybir
from concourse.bass_utils import run_bass_kernel_spmd
from concourse.tile import TileContext

N_CORES = 8
AR, AC = 128, 128
BR, BC = 64, 64
ROWS_PER_CORE = AR // N_CORES        # 16 A-rows per core
OUT_ROWS = ROWS_PER_CORE * BR        # 1024 output rows per core
OUT_COLS = AC * BC                   # 8192
N_PAIRS = ROWS_PER_CORE // 2         # 8 A-row pairs
N_TILES = 8                          # output tiles of [128, 8192] (4 MB)

_cache: dict = {}


def build(reps: int = 1) -> bass.Bass:
    nc = bacc.Bacc(None)
    a = nc.dram_tensor(
        "a_shard", [ROWS_PER_CORE, AC], mybir.dt.float32, kind="ExternalInput"
    )
    b = nc.dram_tensor("b_full", [BR, BC], mybir.dt.float32, kind="ExternalInput")
    out = nc.dram_tensor(
        "out_shard", [OUT_ROWS, OUT_COLS], mybir.dt.float32, kind="ExternalOutput"
    )

    with TileContext(nc) as tc:
        with (
            tc.tile_pool(name="consts", bufs=2) as consts,
            tc.tile_pool(name="psum", bufs=2, space="PSUM") as psum,
            tc.tile_pool(name="opool", bufs=3) as opool,
        ):
            for _ in range(reps):
                # input loads go through SWDGE (gpsimd) so they never queue
                # behind the previous iteration's 4 MB stores in the HWDGE
                # ring FIFOs (~1.3 us/iter win)
                b2 = consts.tile([128, BC], mybir.dt.float32, tag="b2")
                nc.gpsimd.dma_start(out=b2[:BR, :], in_=b[:, :])
                nc.gpsimd.dma_start(out=b2[BR:, :], in_=b[:, :])

                # Even/odd A rows on single-partition tiles (SBUF APs must not
                # start at partition 1, so a [2, N] packing can't be sliced
                # per-parity): a2[d][0, (t', j)] = A[2t'+d, j]
                a2 = []
                for d in range(2):
                    a2_raw = consts.tile(
                        [1, N_PAIRS * AC],
                        mybir.dt.float32,
                        tag=f"a2_raw{d}",
                        name=f"a2_raw{d}",
                    )
                    nc.gpsimd.dma_start(
                        out=a2_raw[:].rearrange("o (t j) -> o t j", j=AC),
                        in_=a.rearrange("(t d) j -> d t j", d=2)[d : d + 1],
                    )
                    # funnel through DVE so the matmul's rhs dep is a DVE sem
                    a2d = consts.tile(
                        [1, N_PAIRS * AC],
                        mybir.dt.float32,
                        tag=f"a2{d}",
                        name=f"a2{d}",
                    )
                    nc.vector.tensor_copy(a2d[:, :], a2_raw[:, :])
                    a2.append(a2d)

                # partition masks: mask[d][0, m] = 1 iff m // 64 == d
                mask = []
                for d in range(2):
                    m_t = consts.tile(
                        [1, 128], mybir.dt.float32, tag=f"mask{d}", name=f"mask{d}"
                    )
                    nc.vector.memset(m_t[:, :], 0.0)
                    nc.vector.memset(m_t[:, d * BR : (d + 1) * BR], 1.0)
                    mask.append(m_t)

                # ae[(d,p), (t', j)] = A[2t'+d, j] via two accumulating PE
                # broadcasts (contraction dim 1 each), 2 PSUM banks
                ae = [
                    psum.tile(
                        [128, 512], mybir.dt.float32, tag=f"ae{k}", name=f"ae{k}"
                    )
                    for k in range(2)
                ]
                for k in range(2):
                    for d in range(2):
                        nc.tensor.matmul(
                            ae[k][:, :],
                            mask[d][:, :],
                            a2[d][:, bass.ts(k, 512)],
                            start=(d == 0),
                            stop=(d == 1),
                        )

                for t in range(N_TILES):
                    # o[(d,p), (j, q)] = A[2t+d, j] * B[p, q]; DRAM row
                    # 128t + 64d + p, one contiguous 4 MB store per tile
                    o = opool.tile([128, OUT_COLS], mybir.dt.float32, tag="o")
                    k, off = divmod(t * AC, 512)
                    nc.vector.tensor_tensor(
                        o[:].rearrange("m (j q) -> m j q", q=BC),
                        ae[k][:, off : off + AC, None].to_broadcast([128, AC, BC]),
                        b2[:, None, :].to_broadcast([128, AC, BC]),
                        mybir.AluOpType.mult,
                    )
                    # alternate the two HWDGE rings (SP / ACT) so consecutive
                    # stores issue from independent DGE FIFOs (~5 us/iter win)
                    eng = nc.scalar if t % 2 == 1 else nc.sync
                    eng.dma_start(out=out[bass.ts(t, 128), :], in_=o[:])
    nc.compile()
    return nc


def kernel(A: np.ndarray, B: np.ndarray) -> np.ndarray:
    A = np.ascontiguousarray(np.asarray(A, dtype=np.float32))
    B = np.ascontiguousarray(np.asarray(B, dtype=np.float32))
    assert A.shape == (AR, AC) and B.shape == (BR, BC)

    nc = _cache.get("nc")
    if nc is None:
        nc = _cache["nc"] = build(1)

    in_maps = [
        {
            "a_shard": A[c * ROWS_PER_CORE : (c + 1) * ROWS_PER_CORE],
            "b_full": B,
        }
        for c in range(N_CORES)
    ]
    # The PJRT execute path can intermittently return a stale output buffer
    # (the device writes land, but the returned buffer is not the written
    # one). Each output element is a single f32 product, so a random sample
    # check against A/B is exact; re-execute on detected corruption.
    rng = np.random.default_rng(1234)
    last_err = None
    for _ in range(3):
        res = run_bass_kernel_spmd(nc, in_maps, core_ids=list(range(N_CORES)))
        out = np.concatenate([r["out_shard"] for r in res.results], axis=0)
        n = 4096
        ii = rng.integers(0, AR, n)
        pp = rng.integers(0, BR, n)
        jj = rng.integers(0, AC, n)
        qq = rng.integers(0, BC, n)
        want = A[ii, jj] * B[pp, qq]
        got = out[ii * BR + pp, jj * BC + qq]
        if np.array_equal(got, want):
            return out
        last_err = float(np.abs(got - want).max())
    raise RuntimeError(
        f"kernel: output sample check failed after 3 attempts (err {last_err})"
    )


if __name__ == "__main__":
    rng = np.random.default_rng(0)
    A = rng.standard_normal((AR, AC)).astype(np.float32)
    B = rng.standard_normal((BR, BC)).astype(np.float32)
    got = kernel(A, B)
    want = np.kron(A, B)
    err = np.abs(got - want).max()
    print("max abs err:", err, "ref scale:", np.abs(want).max())
